# revision 16
# baseline (speedup 1.0000x reference)
"""Masked fractional Hamming distance over 31 circular rotations, on 8 trn2 cores.

Math: for shift s, num(s)/den(s) with
  den(s) = sum maskbits = corr(ma, mb)(2s)        (l,k fused -> lag 2s)
  num(s) = masked differing bits; with the sign-encode
  A = (ia<<7)|ma, B = (ib<<7)|mb read as fp8e4m3 the bytes become
  {+0, -0, +2^-9, -2^-9} (sign=iris, magnitude=mask), so
  corr(A, B)(2s) = (den - 2*num) * 2^-18, corr(ma, mb raw bytes) = den * 2^-18.
Both correlations are computed as banded matmuls on the PE: contraction over
rows (128/partition group), stationary = 128-column chunk of the A side,
moving = 188-column window of the (30-halo-padded) B side; every chunk and
row-group accumulates into one (128,188) PSUM tile per pair since the
diagonal offset d = j - i - 30 is tiling-invariant. Band diagonals are summed
on the host (exact integers scaled by 2^-18).
"""

import numpy as np

N_CORES = 8
B_FULL, L = 4096, 2048
R = 15
J = 2 * L                      # fused (l, k) axis, circular shifts = even lags
B_SH = B_FULL // N_CORES       # 512 batches per core
ROWS = 2 * B_SH                # 1024 rows of length J per core
HALO = 2 * R                   # 30
NW = 128 + 2 * HALO            # 188 moving window
N_GROUPS = ROWS // 128         # 8
N_CHUNKS = J // 128            # 32

_CACHE = {}


def _build_program():
    import concourse.bass as bass
    import concourse.tile as tile
    from concourse import bacc, mybir

    u8 = mybir.dt.uint8
    u16 = mybir.dt.uint16
    f8 = mybir.dt.float8e4
    f32 = mybir.dt.float32
    Alu = mybir.AluOpType

    nc = bass.Bass()
    ia_d = nc.declare_dram_parameter("ia", [ROWS, J], u8, isOutput=False)
    ma_d = nc.declare_dram_parameter("ma", [ROWS, J], u8, isOutput=False)
    ib_d = nc.declare_dram_parameter("ib", [ROWS, J], u8, isOutput=False)
    mb_d = nc.declare_dram_parameter("mb", [ROWS, J], u8, isOutput=False)
    out_d = nc.declare_dram_parameter("out", [2, 128, NW], f32, isOutput=True)

    with tile.TileContext(nc) as tc:
        with (
            tc.tile_pool(name="raw", bufs=2) as raw_pool,
            tc.tile_pool(name="enc", bufs=2) as enc_pool,
            tc.tile_pool(name="acc", bufs=1, space="PSUM") as psum_pool,
        ):
            ps_ab = psum_pool.tile([128, NW], f32)
            ps_mm = psum_pool.tile([128, NW], f32)

            for g in range(N_GROUPS):
                rows = slice(g * 128, (g + 1) * 128)
                ia_t = raw_pool.tile([128, J], u8, tag="ia")
                ma_t = raw_pool.tile([128, J], u8, tag="ma")
                ib_t = raw_pool.tile([128, J], u8, tag="ib")
                mb_t = raw_pool.tile([128, J], u8, tag="mb")
                a_t = enc_pool.tile([128, J], u8, tag="A")
                b_t = enc_pool.tile([128, J + 2 * HALO], u8, tag="B")
                am_t = enc_pool.tile([128, J], u8, tag="AM")
                bm_t = enc_pool.tile([128, J + 2 * HALO], u8, tag="BM")

                # Single-writer dataflow: raw tiles are DMA-written only,
                # derived tiles are DVE-written only (Tile's semaphore pass
                # is not transitive, and compute ISA structs hold ONE wait).
                nc.sync.dma_start(ia_t[:], ia_d[rows, :])
                nc.sync.dma_start(ma_t[:], ma_d[rows, :])
                nc.sync.dma_start(ib_t[:], ib_d[rows, :])
                nc.sync.dma_start(mb_t[:], mb_d[rows, :])

                # Tiny DVE reads that "observe" each DMA first, so the real
                # DVE ops below carry at most their same-engine wait.
                scr = raw_pool.tile([128, 16], u8, tag="scr")
                nc.vector.tensor_copy(scr[:, 0:4], ia_t[:, 0:4])
                nc.vector.tensor_copy(scr[:, 4:8], ma_t[:, 0:4])
                nc.vector.tensor_copy(scr[:, 8:12], ib_t[:, 0:4])
                nc.vector.tensor_copy(scr[:, 12:16], mb_t[:, 0:4])

                # A = (ia << 7) | ma per byte, done on u16-viewed data (both
                # bytes of a pair are {0,1}: the shift never crosses bytes).
                nc.vector.tensor_scalar_mul(
                    a_t[:].bitcast(u16), ia_t[:].bitcast(u16), 128.0
                )
                nc.vector.tensor_tensor(
                    a_t[:].bitcast(u16),
                    a_t[:].bitcast(u16),
                    ma_t[:].bitcast(u16),
                    op=Alu.bitwise_or,
                )
                nc.vector.tensor_scalar_mul(
                    b_t[:, HALO : HALO + J].bitcast(u16),
                    ib_t[:].bitcast(u16),
                    128.0,
                )
                nc.vector.tensor_tensor(
                    b_t[:, HALO : HALO + J].bitcast(u16),
                    b_t[:, HALO : HALO + J].bitcast(u16),
                    mb_t[:].bitcast(u16),
                    op=Alu.bitwise_or,
                )
                # circular halos for the encoded moving tile
                nc.vector.tensor_copy(b_t[:, 0:HALO], b_t[:, J : J + HALO])
                nc.vector.tensor_copy(b_t[:, HALO + J :], b_t[:, HALO : 2 * HALO])

                # DVE-written copies of the masks, so the mask matmuls also
                # need only the DVE semaphore (raw tiles stay DMA-only).
                nc.vector.tensor_copy(am_t[:].bitcast(u16), ma_t[:].bitcast(u16))
                nc.vector.tensor_copy(
                    bm_t[:, HALO : HALO + J].bitcast(u16), mb_t[:].bitcast(u16)
                )
                nc.vector.tensor_copy(bm_t[:, 0:HALO], bm_t[:, J : J + HALO])
                nc.vector.tensor_copy(bm_t[:, HALO + J :], bm_t[:, HALO : 2 * HALO])

                for c in range(N_CHUNKS):
                    a0 = c * 128
                    first = g == 0 and c == 0
                    last = g == N_GROUPS - 1 and c == N_CHUNKS - 1
                    nc.tensor.matmul(
                        ps_ab[:],
                        a_t[:, a0 : a0 + 128].bitcast(f8),
                        b_t[:, a0 : a0 + NW].bitcast(f8),
                        start=first,
                        stop=last,
                    )
                    nc.tensor.matmul(
                        ps_mm[:],
                        am_t[:, a0 : a0 + 128].bitcast(f8),
                        bm_t[:, a0 : a0 + NW].bitcast(f8),
                        start=first,
                        stop=last,
                    )

            out_sb = enc_pool.tile([128, 2, NW], f32, tag="out")
            nc.vector.tensor_copy(out_sb[:, 0], ps_ab[:])
            nc.vector.tensor_copy(out_sb[:, 1], ps_mm[:])
            nc.sync.dma_start(out_d[0], out_sb[:, 0])
            nc.sync.dma_start(out_d[1], out_sb[:, 1])

    import bass_rust as _bass_rust

    _bass_rust.move_matmul_waits_to_ldweights(nc.m)
    _bass_rust.generate_event_semaphores(nc)
    return nc


def _get_program():
    if "nc" not in _CACHE:
        _CACHE["nc"] = _build_program()
    return _CACHE["nc"]


def _shard(x):
    x = np.asarray(x)
    if x.dtype != np.uint8:
        x = x.view(np.uint8) if x.dtype == np.bool_ else x.astype(np.uint8)
    return [
        np.ascontiguousarray(x[:, c * B_SH : (c + 1) * B_SH]).reshape(ROWS, J)
        for c in range(N_CORES)
    ]


def kernel(iris_codes_a, mask_codes_a, iris_codes_b, mask_codes_b, _trace=False):
    from concourse.bass_utils import run_bass_kernel_spmd

    nc = _get_program()
    shards = {
        "ia": _shard(iris_codes_a),
        "ma": _shard(mask_codes_a),
        "ib": _shard(iris_codes_b),
        "mb": _shard(mask_codes_b),
    }
    in_maps = [{k: v[c] for k, v in shards.items()} for c in range(N_CORES)]
    res = run_bass_kernel_spmd(nc, in_maps, list(range(N_CORES)), trace=_trace)
    _CACHE["last_result"] = res

    acc = np.zeros((2, 128, NW), np.float64)
    for r in res.results:
        acc += r["out"].astype(np.float64)

    shifts = np.arange(-R, R + 1)
    cab = np.array([np.trace(acc[0], offset=HALO + 2 * s) for s in shifts])
    den = np.array([np.trace(acc[1], offset=HALO + 2 * s) for s in shifts])
    cab = np.rint(cab * 2.0**18)
    den = np.rint(den * 2.0**18)
    num = (den - cab) / 2.0
    dist = num.astype(np.float32) / den.astype(np.float32)
    out = np.minimum(np.float32(1.0), dist.min())
    return np.asarray([out], dtype=np.float32)


# revision 17
# speedup vs baseline: 1.0480x; 1.0480x over previous
"""Masked fractional Hamming distance over 31 circular rotations, on 8 trn2 cores.

Math: for shift s, num(s)/den(s) with
  den(s) = sum maskbits = corr(ma, mb)(2s)        (l,k fused -> lag 2s)
  num(s) = masked differing bits; with the sign-encode
  A = (ia<<7)|ma, B = (ib<<7)|mb read as fp8e4m3 the bytes become
  {+0, -0, +2^-9, -2^-9} (sign=iris, magnitude=mask), so
  corr(A, B)(2s) = (den - 2*num) * 2^-18, corr(ma, mb raw bytes) = den * 2^-18.
Both correlations are computed as banded matmuls on the PE: contraction over
rows (128/partition group), stationary = 128-column chunk of the A side,
moving = 188-column window of the (30-halo-padded) B side; every chunk and
row-group accumulates into one (128,188) PSUM tile per pair since the
diagonal offset d = j - i - 30 is tiling-invariant. Band diagonals are summed
on the host (exact integers scaled by 2^-18).
"""

import numpy as np

N_CORES = 8
B_FULL, L = 4096, 2048
R = 15
J = 2 * L                      # fused (l, k) axis, circular shifts = even lags
B_SH = B_FULL // N_CORES       # 512 batches per core
ROWS = 2 * B_SH                # 1024 rows of length J per core
HALO = 2 * R                   # 30
NW = 128 + 2 * HALO            # 188 moving window
N_GROUPS = ROWS // 128         # 8
N_CHUNKS = J // 128            # 32

_CACHE = {}


def _build_program():
    import concourse.bass as bass
    import concourse.tile as tile
    from concourse import bacc, mybir

    u8 = mybir.dt.uint8
    u16 = mybir.dt.uint16
    f8 = mybir.dt.float8e4
    f32 = mybir.dt.float32
    Alu = mybir.AluOpType

    nc = bass.Bass()
    ia_d = nc.declare_dram_parameter("ia", [ROWS, J], u8, isOutput=False)
    ma_d = nc.declare_dram_parameter("ma", [ROWS, J], u8, isOutput=False)
    ib_d = nc.declare_dram_parameter("ib", [ROWS, J], u8, isOutput=False)
    mb_d = nc.declare_dram_parameter("mb", [ROWS, J], u8, isOutput=False)
    out_d = nc.declare_dram_parameter("out", [2, 128, NW], f32, isOutput=True)

    with tile.TileContext(nc) as tc:
        with (
            tc.tile_pool(name="raw", bufs=3) as raw_pool,
            tc.tile_pool(name="enc", bufs=3) as enc_pool,
            tc.tile_pool(name="acc", bufs=1, space="PSUM") as psum_pool,
        ):
            ps_ab = psum_pool.tile([128, NW], f32)
            ps_mm = psum_pool.tile([128, NW], f32)

            for g in range(N_GROUPS):
                rows = slice(g * 128, (g + 1) * 128)
                ia_t = raw_pool.tile([128, J], u8, tag="ia")
                ma_t = raw_pool.tile([128, J], u8, tag="ma")
                ib_t = raw_pool.tile([128, J], u8, tag="ib")
                mb_t = raw_pool.tile([128, J + 2 * HALO], u8, tag="mb")
                a_t = enc_pool.tile([128, J], u8, tag="A")
                b_t = enc_pool.tile([128, J + 2 * HALO], u8, tag="B")

                nc.sync.dma_start(ia_t[:], ia_d[rows, :])
                nc.sync.dma_start(ma_t[:], ma_d[rows, :])
                nc.sync.dma_start(ib_t[:], ib_d[rows, :])
                nc.sync.dma_start(mb_t[:, HALO : HALO + J], mb_d[rows, :])

                # A = (ia << 7) | ma per byte, done on u16-viewed data (both
                # bytes of a pair are {0,1}: the shift never crosses bytes).
                nc.vector.tensor_scalar_mul(
                    a_t[:].bitcast(u16), ia_t[:].bitcast(u16), 128.0
                )
                nc.vector.tensor_tensor(
                    a_t[:].bitcast(u16),
                    a_t[:].bitcast(u16),
                    ma_t[:].bitcast(u16),
                    op=Alu.bitwise_or,
                )
                nc.vector.tensor_scalar_mul(
                    b_t[:, HALO : HALO + J].bitcast(u16),
                    ib_t[:].bitcast(u16),
                    128.0,
                )
                nc.vector.tensor_tensor(
                    b_t[:, HALO : HALO + J].bitcast(u16),
                    b_t[:, HALO : HALO + J].bitcast(u16),
                    mb_t[:, HALO : HALO + J].bitcast(u16),
                    op=Alu.bitwise_or,
                )
                # circular halos for the moving-side tiles
                nc.vector.tensor_copy(b_t[:, 0:HALO], b_t[:, J : J + HALO])
                nc.vector.tensor_copy(b_t[:, HALO + J :], b_t[:, HALO : 2 * HALO])
                nc.vector.tensor_copy(mb_t[:, 0:HALO], mb_t[:, J : J + HALO])
                nc.vector.tensor_copy(mb_t[:, HALO + J :], mb_t[:, HALO : 2 * HALO])

                for c in range(N_CHUNKS):
                    a0 = c * 128
                    first = g == 0 and c == 0
                    last = g == N_GROUPS - 1 and c == N_CHUNKS - 1
                    nc.tensor.matmul(
                        ps_ab[:],
                        a_t[:, a0 : a0 + 128].bitcast(f8),
                        b_t[:, a0 : a0 + NW].bitcast(f8),
                        start=first,
                        stop=last,
                    )
                    nc.tensor.matmul(
                        ps_mm[:],
                        ma_t[:, a0 : a0 + 128].bitcast(f8),
                        mb_t[:, a0 : a0 + NW].bitcast(f8),
                        start=first,
                        stop=last,
                    )

            out_sb = enc_pool.tile([128, 2, NW], f32, tag="out")
            nc.vector.tensor_copy(out_sb[:, 0], ps_ab[:])
            nc.vector.tensor_copy(out_sb[:, 1], ps_mm[:])
            nc.sync.dma_start(out_d[0], out_sb[:, 0])
            nc.sync.dma_start(out_d[1], out_sb[:, 1])

    import bass_rust as _bass_rust

    _bass_rust.move_matmul_waits_to_ldweights(nc.m)
    _bass_rust.generate_event_semaphores(nc)
    return nc


def _get_program():
    if "nc" not in _CACHE:
        _CACHE["nc"] = _build_program()
    return _CACHE["nc"]


def _shard(x):
    x = np.asarray(x)
    if x.dtype != np.uint8:
        x = x.view(np.uint8) if x.dtype == np.bool_ else x.astype(np.uint8)
    return [
        np.ascontiguousarray(x[:, c * B_SH : (c + 1) * B_SH]).reshape(ROWS, J)
        for c in range(N_CORES)
    ]


def kernel(iris_codes_a, mask_codes_a, iris_codes_b, mask_codes_b, _trace=False):
    from concourse.bass_utils import run_bass_kernel_spmd

    nc = _get_program()
    shards = {
        "ia": _shard(iris_codes_a),
        "ma": _shard(mask_codes_a),
        "ib": _shard(iris_codes_b),
        "mb": _shard(mask_codes_b),
    }
    in_maps = [{k: v[c] for k, v in shards.items()} for c in range(N_CORES)]
    res = run_bass_kernel_spmd(nc, in_maps, list(range(N_CORES)), trace=_trace)
    _CACHE["last_result"] = res

    acc = np.zeros((2, 128, NW), np.float64)
    for r in res.results:
        acc += r["out"].astype(np.float64)

    shifts = np.arange(-R, R + 1)
    cab = np.array([np.trace(acc[0], offset=HALO + 2 * s) for s in shifts])
    den = np.array([np.trace(acc[1], offset=HALO + 2 * s) for s in shifts])
    cab = np.rint(cab * 2.0**18)
    den = np.rint(den * 2.0**18)
    num = (den - cab) / 2.0
    dist = num.astype(np.float32) / den.astype(np.float32)
    out = np.minimum(np.float32(1.0), dist.min())
    return np.asarray([out], dtype=np.float32)
